# revision 18
# baseline (speedup 1.0000x reference)
"""Trainium2 Bass kernel for nn_Distance_kernel (histogram_binning).

Self-contained: takes FULL inputs, shards batch across 8 NeuronCores,
runs a Bass/Tile kernel per core, gathers the full outputs.

Per-core plan (bs = 16 batch rows):
  embed (cos/sin with split-k Cody-Waite range reduction + ACT Sin)
  -> 3-layer MLP (fp32 PE matmuls, LN via DVE + NR-rsqrt, ACT tanh)
  -> histogram gathers as one-hot QUADRANT matmuls (exact 4-fold symmetry
     of the radial bin map), mirror-expanded to full maps with
     negative-stride DVE copies, streamed out with large DMAs.
"""
import numpy as np

import concourse.bacc as bacc
import concourse.mybir as mybir
from concourse.tile import TileContext
from concourse.bass_utils import run_bass_kernel_spmd

F32 = mybir.dt.float32
F32R = mybir.dt.float32r
I32 = mybir.dt.int32
AF = mybir.ActivationFunctionType
ALU = mybir.AluOpType
AX = mybir.AxisListType

# ---------------- problem constants (hardcoded per spec) ----------------
B = 128
NCORES = 8
BS = B // NCORES            # 16
N_EMBED = 64
FE = 32                     # F_EMBED
H1 = 512
D1 = 768
D2 = 1536
WL = 0.638
PITCH = 8e-06

# (name, L, n_grp, Mg_last, channels) ; rows of output = CH*BS (c-major)
#   c1:  kvb[:, 0:512]    CH=8  L=64
#   c2:  kvb[:, 512:1024] CH=16 L=32
#   c3:  kvb[:, 1024:1536]CH=32 L=16
#   c11: kv[:, 0:256]     CH=4  L=64
#   c22: kv[:, 256:512]   CH=8  L=32
#   c33: kv[:, 512:768]   CH=16 L=16

# ---------------- embed / range-reduction constants ----------------
_TWO_PI = 2.0 * np.pi
_C1 = 6.28125                      # 2pi to ~10 bits (804/128)
_rem = _TWO_PI - _C1
_e = np.frexp(_rem)[1]
_C2 = float(np.round(_rem * 2.0 ** (12 - _e)) * 2.0 ** (_e - 12))
_C3 = float(np.float32(_TWO_PI - _C1 - _C2))
_INV2PI = float(np.float32(1.0 / _TWO_PI))
_MAGIC = float(np.float32(1.5 * 2 ** 23))
_PI_F = float(np.float32(np.pi))
_TWO_PI_F = float(np.float32(_TWO_PI))
_HALF_PI_F = float(np.float32(np.pi / 2))


def _freq_bands():
    """Exact f32 replication of the reference freq_bands computation."""
    wavelength = WL * 1e-06
    min_fre = 2 * np.pi / wavelength * (1 - 2 * (wavelength / PITCH / 2) ** 2) ** 0.5
    max_fre = 2 * np.pi / wavelength
    lin = np.linspace(1.0, FE, FE, dtype=np.float32)
    return (np.float32((max_fre - min_fre) / FE) * lin
            + np.float32(min_fre)).astype(np.float32)


def _idx_full(L):
    ax = np.linspace(-float(L), float(L), 2 * L)
    xg, yg = np.meshgrid(ax, ax, indexing="ij")
    dis = np.sqrt(xg ** 2 + yg ** 2)
    interval = dis.max() / L
    idx = np.floor(dis / (interval + 1e-4)).astype(np.int64)
    return idx


def _eq_mat(L):
    """One-hot expansion matrix for the positive quadrant: (L, L*L) f32."""
    idx = _idx_full(L)
    q = idx[L:, L:].ravel()
    assert q.max() < L
    E = np.zeros((L, L * L), np.float32)
    E[q, np.arange(L * L)] = 1.0
    return E


# ---------------- kernel builder ----------------

def _build():
    nc = bacc.Bacc("TRN2", target_bir_lowering=False, debug=False)

    # inputs
    xb_d = nc.declare_dram_parameter("xb", [64, BS], F32, isOutput=False)
    fr_d = nc.declare_dram_parameter("freqs", [64, 1], F32, isOutput=False)
    w1_d = nc.declare_dram_parameter("W1", [64, H1], F32R, isOutput=False)
    w2_d = nc.declare_dram_parameter("W2", [H1, D1], F32R, isOutput=False)
    w3_d = nc.declare_dram_parameter("W3", [D1, D2], F32R, isOutput=False)
    gb_ds = {}
    for nm_, F_ in (("g1p", H1), ("be1p", H1), ("g2p", D1), ("be2p", D1),
                    ("g3p", D2), ("be3p", D2)):
        gb_ds[nm_] = nc.declare_dram_parameter(nm_, [16, F_], F32, isOutput=False)
    bp_d = nc.declare_dram_parameter("biaspack", [1, 16 + H1 + D1 + D2], F32R, isOutput=False)
    id_d = nc.declare_dram_parameter("ident", [16, 16], F32, isOutput=False)
    eq1_d = nc.declare_dram_parameter("eq1", [64, 64 * 64], F32R, isOutput=False)
    eq2_d = nc.declare_dram_parameter("eq2", [32, 32 * 32], F32R, isOutput=False)
    eq3_d = nc.declare_dram_parameter("eq3", [16, 16 * 16], F32R, isOutput=False)

    # outputs: rows are (channel*BS + b), cols are flattened (2L, 2L)
    c1_d = nc.declare_dram_parameter("c1o", [128, 16384], F32, isOutput=True)
    c2_d = nc.declare_dram_parameter("c2o", [256, 4096], F32, isOutput=True)
    c3_d = nc.declare_dram_parameter("c3o", [512, 1024], F32, isOutput=True)
    c11_d = nc.declare_dram_parameter("c11o", [64, 16384], F32, isOutput=True)
    c22_d = nc.declare_dram_parameter("c22o", [128, 4096], F32, isOutput=True)
    c33_d = nc.declare_dram_parameter("c33o", [256, 1024], F32, isOutput=True)

    with TileContext(nc) as tc:
        with (
            tc.tile_pool(name="consts", bufs=1) as cp,
            tc.tile_pool(name="acts", bufs=1) as ap,
            tc.tile_pool(name="wch", bufs=3) as wp,
            tc.tile_pool(name="stage", bufs=4) as sp,
            tc.tile_pool(name="mlp_ps", bufs=1, space="PSUM") as mlp_ps,
            tc.tile_pool(name="tr_ps", bufs=1, space="PSUM") as tr_ps,
            tc.tile_pool(name="g_ps", bufs=2, space="PSUM") as g_ps,
        ):
            # ---- constant loads ----
            xb = cp.tile([64, BS], F32, name="xb_t")
            fr = cp.tile([64, 1], F32, name="fr_t")
            w1 = cp.tile([64, H1], F32R, name="w1_t")
            gb_ts = {}
            for nm_, F_ in (("g1p", H1), ("be1p", H1), ("g2p", D1), ("be2p", D1),
                            ("g3p", D2), ("be3p", D2)):
                gb_ts[nm_] = cp.tile([16, F_], F32, name=nm_ + "_t")
            bp = cp.tile([1, 16 + H1 + D1 + D2], F32R, name="bp_t")
            idt = cp.tile([16, 16], F32, name="idt_t")
            eq1 = cp.tile([64, 64 * 64], F32R, name="eq1_t")
            eq2 = cp.tile([32, 32 * 32], F32R, name="eq2_t")
            eq3 = cp.tile([16, 16 * 16], F32R, name="eq3_t")
            nc.sync.dma_start(out=xb[:], in_=xb_d[:])
            nc.sync.dma_start(out=fr[:], in_=fr_d[:])
            nc.sync.dma_start(out=w1[:], in_=w1_d[:])
            for nm_ in gb_ts:
                nc.sync.dma_start(out=gb_ts[nm_][:], in_=gb_ds[nm_][:])
            nc.sync.dma_start(out=bp[:], in_=bp_d[:])
            nc.sync.dma_start(out=idt[:], in_=id_d[:])

            ones_ap = bp[:, 0:16]                       # (1,16)
            b1_ap = bp[:, 16:16 + H1]
            b2_ap = bp[:, 16 + H1:16 + H1 + D1]
            b3_ap = bp[:, 16 + H1 + D1:16 + H1 + D1 + D2]
            g1_ap, be1_ap = gb_ts["g1p"][:], gb_ts["be1p"][:]
            g2_ap, be2_ap = gb_ts["g2p"][:], gb_ts["be2p"][:]
            g3_ap, be3_ap = gb_ts["g3p"][:], gb_ts["be3p"][:]

            # ---- embed: t = x*f (exact f32), reduce mod 2pi, sin table ----
            t = ap.tile([64, BS], F32, name="t_emb")
            nc.vector.tensor_scalar(t[:], xb[:], fr[:, 0:1], None, ALU.mult)
            k = ap.tile([64, BS], F32, name="k_emb")
            nc.vector.tensor_scalar(k[:], t[:], _INV2PI, None, ALU.mult)
            nc.vector.tensor_scalar(k[:], k[:], _MAGIC, _MAGIC, ALU.add, ALU.subtract)
            k1 = ap.tile([64, BS], F32, name="k1_emb")
            nc.vector.tensor_scalar(k1[:], k[:], 2.0 ** -11, _MAGIC, ALU.mult, ALU.add)
            nc.vector.tensor_scalar(k1[:], k1[:], _MAGIC, 2048.0, ALU.subtract, ALU.mult)
            k2 = ap.tile([64, BS], F32, name="k2_emb")
            nc.vector.tensor_sub(k2[:], k[:], k1[:])
            r = ap.tile([64, BS], F32, name="r_emb")
            nc.vector.cody_waite_cascade(r[:], t[:], k1[:], _C1, _C2, 0.0)
            nc.vector.cody_waite_cascade(r[:], r[:], k2[:], _C1, _C2, 0.0)
            nc.vector.cody_waite_cascade(r[:], r[:], k[:], _C3, 0.0, 0.0)
            yc = ap.tile([64, BS], F32, name="yc_emb")
            ys = ap.tile([64, BS], F32, name="ys_emb")
            nc.vector.add_range_wrap(yc[:], r[:], _HALF_PI_F, _PI_F, _TWO_PI_F)
            nc.vector.add_range_wrap(ys[:], r[:], 0.0, _PI_F, _TWO_PI_F)
            dT = ap.tile([64, BS], F32, name="dT")
            nc.scalar.activation(dT[0:32, :], yc[0:32, :], AF.Sin)
            nc.scalar.activation(dT[32:64, :], ys[32:64, :], AF.Sin)
            dTr = ap.tile([64, BS], F32R, name="dTr")
            nc.vector.tensor_copy(dTr[:], dT[:])

            # ---- helpers ----
            def rsqrt_nr(nm, vv):
                """rsqrt of (16,1) tile vv (v+eps already applied), ~1ulp."""
                gi = ap.tile([16, 1], I32, name=f"gi_{nm}")
                mg = ap.tile([16, 1], I32, name=f"mg_{nm}")
                nc.vector.memset(mg[:], 0x5F3759DF)
                nc.vector.tensor_scalar(gi[:], vv[:].bitcast(I32), 1, None,
                                        ALU.logical_shift_right)
                nc.vector.tensor_sub(gi[:], mg[:], gi[:])
                g = gi[:].bitcast(F32)
                tt = ap.tile([16, 1], F32, name=f"tt_{nm}")
                for _ in range(3):
                    nc.vector.tensor_mul(tt[:], g, g)
                    nc.vector.tensor_mul(tt[:], tt[:], vv[:])
                    nc.vector.tensor_scalar(tt[:], tt[:], -0.5, 1.5, ALU.mult, ALU.add)
                    nc.vector.tensor_mul(g, g, tt[:])
                return gi

            def ln_tanh(nm, pin, F, g_ap, be_ap):
                """LayerNorm(+affine)+tanh of PSUM (16,F) -> SBUF tile."""
                s = ap.tile([16, 1], F32, name=f"s_{nm}")
                scr = ap.tile([16, F], F32, name=f"scr_{nm}")
                nc.scalar.activation(scr[:], pin, AF.Identity, accum_out=s[:])
                nm_t = ap.tile([16, 1], F32, name=f"nm_{nm}")
                nc.vector.tensor_scalar(nm_t[:], s[:], -1.0 / F, None, ALU.mult)
                xc = ap.tile([16, F], F32, name=f"xc_{nm}")
                nc.scalar.activation(xc[:], pin, AF.Identity, bias=nm_t[:, 0:1])
                sq = ap.tile([16, F], F32, name=f"sq_{nm}")
                v = ap.tile([16, 1], F32, name=f"v_{nm}")
                nc.scalar.activation(sq[:], xc[:], AF.Square, accum_out=v[:])
                nc.vector.tensor_scalar(v[:], v[:], 1.0 / F, 1e-5, ALU.mult, ALU.add)
                rs = rsqrt_nr(nm, v)
                t2 = ap.tile([16, F], F32, name=f"t2_{nm}")
                nc.scalar.activation(t2[:], xc[:], AF.Identity,
                                     scale=rs[:, 0:1].bitcast(F32))
                nc.vector.tensor_mul(t2[:], t2[:], g_ap)
                nc.vector.tensor_add(t2[:], t2[:], be_ap)
                out = ap.tile([16, F], F32, name=f"act_{nm}")
                nc.scalar.activation(out[:], t2[:], AF.Tanh)
                return out

            def transpose_pack(nm, src, col_blocks, p_out, dt_out=F32):
                """Transpose (16, p_out) col blocks of src into one SBUF tile.

                col_blocks: list of (col_offset,) source column starts, each
                block is (16, p_out) -> out (p_out, 16) written at free
                offset i*16. Returns SBUF tile (p_out, 16*len(col_blocks)).
                """
                n = len(col_blocks)
                pt = tr_ps.tile([p_out, 16 * n], F32, name=f"pt_{nm}", tag="trp")
                for i, co in enumerate(col_blocks):
                    nc.tensor.transpose(pt[:, i * 16:(i + 1) * 16],
                                        src[:, co:co + p_out], idt[:])
                st = ap.tile([p_out, 16 * n], dt_out, name=f"st_{nm}")
                nc.vector.tensor_copy(st[:], pt[:])
                return st

            # ---- layer 1 ----
            ph = mlp_ps.tile([16, H1], F32, name="ph", tag="mlp")
            nc.tensor.matmul(ph[:], dTr[:], w1[:], start=True, stop=False)
            nc.tensor.matmul(ph[:], ones_ap, b1_ap, start=False, stop=True)
            h = ln_tanh("l1", ph[:], H1, g1_ap, be1_ap)

            # ---- layer 2 ----
            hT = transpose_pack("hT", h, [128 * i for i in range(4)], 128, F32R)
            pkv = mlp_ps.tile([16, D1], F32, name="pkv", tag="mlp")
            n2 = ((0, 512), (512, 768))
            for i in range(4):
                w2c = wp.tile([128, D1], F32R, name=f"w2_{i}", tag="wch")
                nc.sync.dma_start(out=w2c[:], in_=w2_d[128 * i:128 * (i + 1), :])
                for ns, ne in n2:
                    nc.tensor.matmul(pkv[:, ns:ne], hT[:, 16 * i:16 * (i + 1)],
                                     w2c[:, ns:ne], start=(i == 0), stop=False)
            for ns, ne in n2:
                nc.tensor.matmul(pkv[:, ns:ne], ones_ap, b2_ap[:, ns:ne],
                                 start=False, stop=True)
            kv = ln_tanh("l2", pkv[:], D1, g2_ap, be2_ap)

            nc.sync.dma_start(out=eq3[:], in_=eq3_d[:])
            nc.sync.dma_start(out=eq2[:], in_=eq2_d[:])
            nc.sync.dma_start(out=eq1[:], in_=eq1_d[:])

            # ---- gather tiles from kv ----
            g11 = transpose_pack("g11", kv, [64 * c for c in range(4)], 64, F32R)
            g22 = transpose_pack("g22", kv, [256 + 32 * c for c in range(8)], 32, F32R)
            g33 = transpose_pack("g33", kv, [512 + 16 * c for c in range(16)], 16, F32R)

            # ---- generic gather + mirror-expansion + store ----
            def gather_out(nm, dram, lhsT, eq, L, grp, eng="v"):
                """One group (<=128 rows) of one output.

                lhsT: (L, Mg) SBUF AP (f32r). eq: (L, L*L) one-hot quadrant
                matrix (f32r). Output rows dram[grp*128:grp*128+Mg].
                PSUM tiles span up to 2 banks (1024 f32) to amortize the
                PSUM->SBUF copy overhead.
                """
                Mg = lhsT.shape[1]
                Q = L * L
                PCH = min(1024, Q)             # psum tile cols (<=2 banks)
                nps = Q // PCH                 # psum tiles per quadrant
                QRp = PCH // L                 # quadrant rows per psum tile
                r0 = grp * 128
                TL = 2 * L

                def copy2(dst, sd):
                    if eng == "v":
                        nc.vector.tensor_copy(dst, sd)
                    else:
                        nc.scalar.activation(dst, sd, AF.Copy)

                def expand(st3, p3, dst_lr0, rows_rev):
                    """4-way mirror copy of one psum tile into staging rows."""
                    if rows_rev:
                        src_dir = p3[:, ::-1, :]
                        src_rev = p3[:, ::-1, ::-1]
                    else:
                        src_dir = p3[:, :, :]
                        src_rev = p3[:, :, ::-1]
                    dsl = st3[:, dst_lr0:dst_lr0 + QRp, :]
                    copy2(dsl[:, :, L:TL], src_dir)
                    copy2(dsl[:, :, 0:L], src_rev)

                for pi in range(nps):
                    pt = g_ps.tile([Mg, PCH], F32, name=f"gp_{nm}_{pi}", tag="gp")
                    for j in range(0, PCH, 512):
                        je = min(PCH, j + 512)
                        nc.tensor.matmul(pt[:, j:je], lhsT,
                                         eq[:, pi * PCH + j:pi * PCH + je],
                                         start=True, stop=True)
                    p3 = pt[:].rearrange("p (r c) -> p r c", r=QRp)
                    qlo = pi * QRp
                    if nps == 1:
                        # whole quadrant in one psum tile: one staging tile
                        st = sp.tile([Mg, 2 * L * TL], F32,
                                     name=f"sg_{nm}", tag="stage")
                        st3 = st[:].rearrange("p (r c) -> p r c", r=2 * L)
                        expand(st3, p3, 0, True)       # top half rows 0..L-1
                        expand(st3, p3, L, False)      # bottom rows L..2L-1
                        nc.sync.dma_start(
                            out=dram[r0:r0 + Mg, 0:2 * L * TL], in_=st[0:Mg, :])
                    else:
                        stb = sp.tile([Mg, QRp * TL], F32,
                                      name=f"sgb_{nm}_{pi}", tag="stage")
                        st3 = stb[:].rearrange("p (r c) -> p r c", r=QRp)
                        expand(st3, p3, 0, False)
                        f0 = L + qlo
                        nc.sync.dma_start(
                            out=dram[r0:r0 + Mg, f0 * TL:(f0 + QRp) * TL],
                            in_=stb[0:Mg, :])
                        stt = sp.tile([Mg, QRp * TL], F32,
                                      name=f"sgt_{nm}_{pi}", tag="stage")
                        st3 = stt[:].rearrange("p (r c) -> p r c", r=QRp)
                        expand(st3, p3, 0, True)
                        f0t = L - qlo - QRp
                        nc.sync.dma_start(
                            out=dram[r0:r0 + Mg, f0t * TL:(f0t + QRp) * TL],
                            in_=stt[0:Mg, :])

            # ---- c11/c22/c33 (from kv) ----
            gather_out("c11", c11_d, g11[:, 0:64], eq1[:], 64, 0)
            gather_out("c22", c22_d, g22[:, 0:128], eq2[:], 32, 0)
            gather_out("c33a", c33_d, g33[:, 0:128], eq3[:], 16, 0, eng="s")
            gather_out("c33b", c33_d, g33[:, 128:256], eq3[:], 16, 1, eng="s")

            # ---- layer 3 ----
            kvT = transpose_pack("kvT", kv, [128 * i for i in range(6)], 128, F32R)
            pkvb = mlp_ps.tile([16, D2], F32, name="pkvb", tag="mlp")
            n3 = ((0, 512), (512, 1024), (1024, 1536))
            for i in range(6):
                w3c = wp.tile([128, D2], F32R, name=f"w3_{i}", tag="wch")
                nc.sync.dma_start(out=w3c[:], in_=w3_d[128 * i:128 * (i + 1), :])
                for ns, ne in n3:
                    nc.tensor.matmul(pkvb[:, ns:ne], kvT[:, 16 * i:16 * (i + 1)],
                                     w3c[:, ns:ne], start=(i == 0), stop=False)
            for ns, ne in n3:
                nc.tensor.matmul(pkvb[:, ns:ne], ones_ap, b3_ap[:, ns:ne],
                                 start=False, stop=True)
            kvb = ln_tanh("l3", pkvb[:], D2, g3_ap, be3_ap)

            # ---- gather tiles from kvb ----
            gc1 = transpose_pack("gc1", kvb, [64 * c for c in range(8)], 64, F32R)
            gc2 = transpose_pack("gc2", kvb, [512 + 32 * c for c in range(16)], 32, F32R)
            gc3 = transpose_pack("gc3", kvb, [1024 + 16 * c for c in range(32)], 16, F32R)

            # ---- c1/c2/c3 ----
            gather_out("c1", c1_d, gc1[:, 0:128], eq1[:], 64, 0)
            gather_out("c2a", c2_d, gc2[:, 0:128], eq2[:], 32, 0, eng="s")
            gather_out("c2b", c2_d, gc2[:, 128:256], eq2[:], 32, 1, eng="s")
            for g in range(4):
                gather_out(f"c3{g}", c3_d, gc3[:, 128 * g:128 * (g + 1)], eq3[:], 16, g, eng="s")

    nc.compile()
    return nc


_CACHE = {}


def _get_nc():
    if "nc" not in _CACHE:
        _CACHE["nc"] = _build()
    return _CACHE["nc"]


def make_in_maps(x, W1, b1, g1, beta1, W2, b2, g2, beta2, W3, b3, g3, beta3):
    f = np.float32
    x = np.asarray(x, f)
    W1 = np.ascontiguousarray(np.asarray(W1, f))
    W2 = np.ascontiguousarray(np.asarray(W2, f))
    W3 = np.ascontiguousarray(np.asarray(W3, f))

    fb = _freq_bands()
    freqs = np.concatenate([fb, fb]).reshape(64, 1).astype(f)

    def rep16(v):
        return np.ascontiguousarray(np.broadcast_to(np.asarray(v, f)[None, :], (16, len(np.asarray(v)))))
    gmaps = {"g1p": rep16(g1), "be1p": rep16(beta1), "g2p": rep16(g2),
             "be2p": rep16(beta2), "g3p": rep16(g3), "be3p": rep16(beta3)}

    bp = np.zeros((1, 16 + H1 + D1 + D2), f)
    bp[0, 0:16] = 1.0
    bp[0, 16:16 + H1] = np.asarray(b1, f)
    bp[0, 16 + H1:16 + H1 + D1] = np.asarray(b2, f)
    bp[0, 16 + H1 + D1:] = np.asarray(b3, f)

    ident = np.eye(16, dtype=f)
    eq1, eq2, eq3 = _eq_mat(64), _eq_mat(32), _eq_mat(16)

    in_maps = []
    for i in range(NCORES):
        xs = x[i * BS:(i + 1) * BS, 0]
        in_maps.append({
            "xb": np.ascontiguousarray(np.broadcast_to(xs[None, :], (64, BS))),
            "freqs": freqs, "W1": W1, "W2": W2, "W3": W3,
            "biaspack": bp, "ident": ident,
            "eq1": eq1, "eq2": eq2, "eq3": eq3, **gmaps,
        })
    return in_maps


def assemble(res):
    def asm(key, CH, L):
        full = np.concatenate(
            [r[key].reshape(CH, BS, 2 * L, 2 * L).transpose(1, 0, 2, 3)
             for r in res], axis=0)
        return np.ascontiguousarray(full)

    return (asm("c1o", 8, 64), asm("c2o", 16, 32), asm("c3o", 32, 16),
            asm("c11o", 4, 64), asm("c22o", 8, 32), asm("c33o", 16, 16))


def kernel(**inputs):
    nc = _get_nc()
    in_maps = make_in_maps(**inputs)
    res = run_bass_kernel_spmd(nc, in_maps, list(range(NCORES))).results
    return assemble(res)


# revision 19
# speedup vs baseline: 1.0298x; 1.0298x over previous
"""Trainium2 Bass kernel for nn_Distance_kernel (histogram_binning).

Self-contained: takes FULL inputs, shards batch across 8 NeuronCores,
runs a Bass/Tile kernel per core, gathers the full outputs.

Per-core plan (bs = 16 batch rows):
  embed (cos/sin with split-k Cody-Waite range reduction + ACT Sin)
  -> 3-layer MLP (fp32 PE matmuls, LN via DVE + NR-rsqrt, ACT tanh)
  -> histogram gathers as one-hot QUADRANT matmuls (exact 4-fold symmetry
     of the radial bin map), mirror-expanded to full maps with
     negative-stride DVE copies, streamed out with large DMAs.
"""
import numpy as np

import concourse.bacc as bacc
import concourse.mybir as mybir
from concourse.tile import TileContext
from concourse.bass_utils import run_bass_kernel_spmd

F32 = mybir.dt.float32
F32R = mybir.dt.float32r
I32 = mybir.dt.int32
AF = mybir.ActivationFunctionType
ALU = mybir.AluOpType
AX = mybir.AxisListType

# ---------------- problem constants (hardcoded per spec) ----------------
B = 128
NCORES = 8
BS = B // NCORES            # 16
N_EMBED = 64
FE = 32                     # F_EMBED
H1 = 512
D1 = 768
D2 = 1536
WL = 0.638
PITCH = 8e-06

# (name, L, n_grp, Mg_last, channels) ; rows of output = CH*BS (c-major)
#   c1:  kvb[:, 0:512]    CH=8  L=64
#   c2:  kvb[:, 512:1024] CH=16 L=32
#   c3:  kvb[:, 1024:1536]CH=32 L=16
#   c11: kv[:, 0:256]     CH=4  L=64
#   c22: kv[:, 256:512]   CH=8  L=32
#   c33: kv[:, 512:768]   CH=16 L=16

# ---------------- embed / range-reduction constants ----------------
_TWO_PI = 2.0 * np.pi
_C1 = 6.28125                      # 2pi to ~10 bits (804/128)
_rem = _TWO_PI - _C1
_e = np.frexp(_rem)[1]
_C2 = float(np.round(_rem * 2.0 ** (12 - _e)) * 2.0 ** (_e - 12))
_C3 = float(np.float32(_TWO_PI - _C1 - _C2))
_INV2PI = float(np.float32(1.0 / _TWO_PI))
_MAGIC = float(np.float32(1.5 * 2 ** 23))
_PI_F = float(np.float32(np.pi))
_TWO_PI_F = float(np.float32(_TWO_PI))
_HALF_PI_F = float(np.float32(np.pi / 2))


def _freq_bands():
    """Exact f32 replication of the reference freq_bands computation."""
    wavelength = WL * 1e-06
    min_fre = 2 * np.pi / wavelength * (1 - 2 * (wavelength / PITCH / 2) ** 2) ** 0.5
    max_fre = 2 * np.pi / wavelength
    lin = np.linspace(1.0, FE, FE, dtype=np.float32)
    return (np.float32((max_fre - min_fre) / FE) * lin
            + np.float32(min_fre)).astype(np.float32)


def _idx_full(L):
    ax = np.linspace(-float(L), float(L), 2 * L)
    xg, yg = np.meshgrid(ax, ax, indexing="ij")
    dis = np.sqrt(xg ** 2 + yg ** 2)
    interval = dis.max() / L
    idx = np.floor(dis / (interval + 1e-4)).astype(np.int64)
    return idx


def _eq_mat(L):
    """One-hot expansion matrix for the positive quadrant: (L, L*L) f32."""
    idx = _idx_full(L)
    q = idx[L:, L:].ravel()
    assert q.max() < L
    E = np.zeros((L, L * L), np.float32)
    E[q, np.arange(L * L)] = 1.0
    return E


# ---------------- kernel builder ----------------

def _build():
    nc = bacc.Bacc("TRN2", target_bir_lowering=False, debug=False)

    # inputs
    xb_d = nc.declare_dram_parameter("xb", [64, BS], F32, isOutput=False)
    fr_d = nc.declare_dram_parameter("freqs", [64, 1], F32, isOutput=False)
    w1_d = nc.declare_dram_parameter("W1", [64, H1], F32R, isOutput=False)
    w2_d = nc.declare_dram_parameter("W2", [H1, D1], F32R, isOutput=False)
    w3_d = nc.declare_dram_parameter("W3", [D1, D2], F32R, isOutput=False)
    gb_ds = {}
    for nm_, F_ in (("g1p", H1), ("be1p", H1), ("g2p", D1), ("be2p", D1),
                    ("g3p", D2), ("be3p", D2)):
        gb_ds[nm_] = nc.declare_dram_parameter(nm_, [16, F_], F32, isOutput=False)
    bp_d = nc.declare_dram_parameter("biaspack", [1, 16 + H1 + D1 + D2], F32R, isOutput=False)
    id_d = nc.declare_dram_parameter("ident", [16, 16], F32, isOutput=False)
    eq1_d = nc.declare_dram_parameter("eq1", [64, 64 * 64], F32R, isOutput=False)
    eq2_d = nc.declare_dram_parameter("eq2", [32, 32 * 32], F32R, isOutput=False)
    eq3_d = nc.declare_dram_parameter("eq3", [16, 16 * 16], F32R, isOutput=False)

    # outputs: rows are (channel*BS + b), cols are flattened (2L, 2L)
    c1_d = nc.declare_dram_parameter("c1o", [128, 16384], F32, isOutput=True)
    c2_d = nc.declare_dram_parameter("c2o", [256, 4096], F32, isOutput=True)
    c3_d = nc.declare_dram_parameter("c3o", [512, 1024], F32, isOutput=True)
    c11_d = nc.declare_dram_parameter("c11o", [64, 16384], F32, isOutput=True)
    c22_d = nc.declare_dram_parameter("c22o", [128, 4096], F32, isOutput=True)
    c33_d = nc.declare_dram_parameter("c33o", [256, 1024], F32, isOutput=True)

    with TileContext(nc) as tc:
        with (
            tc.tile_pool(name="consts", bufs=1) as cp,
            tc.tile_pool(name="acts", bufs=1) as ap,
            tc.tile_pool(name="wch", bufs=3) as wp,
            tc.tile_pool(name="stage", bufs=4) as sp,
            tc.tile_pool(name="mlp_ps", bufs=1, space="PSUM") as mlp_ps,
            tc.tile_pool(name="tr_ps", bufs=1, space="PSUM") as tr_ps,
            tc.tile_pool(name="g_ps", bufs=2, space="PSUM") as g_ps,
        ):
            # ---- constant loads ----
            xb = cp.tile([64, BS], F32, name="xb_t")
            fr = cp.tile([64, 1], F32, name="fr_t")
            w1 = cp.tile([64, H1], F32R, name="w1_t")
            gb_ts = {}
            for nm_, F_ in (("g1p", H1), ("be1p", H1), ("g2p", D1), ("be2p", D1),
                            ("g3p", D2), ("be3p", D2)):
                gb_ts[nm_] = cp.tile([16, F_], F32, name=nm_ + "_t")
            bp = cp.tile([1, 16 + H1 + D1 + D2], F32R, name="bp_t")
            idt = cp.tile([16, 16], F32, name="idt_t")
            eq1 = cp.tile([64, 64 * 64], F32R, name="eq1_t")
            eq2 = cp.tile([32, 32 * 32], F32R, name="eq2_t")
            eq3 = cp.tile([16, 16 * 16], F32R, name="eq3_t")
            nc.sync.dma_start(out=xb[:], in_=xb_d[:])
            nc.sync.dma_start(out=fr[:], in_=fr_d[:])
            nc.sync.dma_start(out=w1[:], in_=w1_d[:])
            for nm_ in gb_ts:
                nc.sync.dma_start(out=gb_ts[nm_][:], in_=gb_ds[nm_][:])
            nc.sync.dma_start(out=bp[:], in_=bp_d[:])
            nc.sync.dma_start(out=idt[:], in_=id_d[:])

            ones_ap = bp[:, 0:16]                       # (1,16)
            b1_ap = bp[:, 16:16 + H1]
            b2_ap = bp[:, 16 + H1:16 + H1 + D1]
            b3_ap = bp[:, 16 + H1 + D1:16 + H1 + D1 + D2]
            g1_ap, be1_ap = gb_ts["g1p"][:], gb_ts["be1p"][:]
            g2_ap, be2_ap = gb_ts["g2p"][:], gb_ts["be2p"][:]
            g3_ap, be3_ap = gb_ts["g3p"][:], gb_ts["be3p"][:]

            # ---- embed: t = x*f (exact f32), reduce mod 2pi, sin table ----
            t = ap.tile([64, BS], F32, name="t_emb")
            nc.vector.tensor_scalar(t[:], xb[:], fr[:, 0:1], None, ALU.mult)
            k = ap.tile([64, BS], F32, name="k_emb")
            nc.vector.tensor_scalar(k[:], t[:], _INV2PI, None, ALU.mult)
            nc.vector.tensor_scalar(k[:], k[:], _MAGIC, _MAGIC, ALU.add, ALU.subtract)
            k1 = ap.tile([64, BS], F32, name="k1_emb")
            nc.vector.tensor_scalar(k1[:], k[:], 2.0 ** -11, _MAGIC, ALU.mult, ALU.add)
            nc.vector.tensor_scalar(k1[:], k1[:], _MAGIC, 2048.0, ALU.subtract, ALU.mult)
            k2 = ap.tile([64, BS], F32, name="k2_emb")
            nc.vector.tensor_sub(k2[:], k[:], k1[:])
            r = ap.tile([64, BS], F32, name="r_emb")
            nc.vector.cody_waite_cascade(r[:], t[:], k1[:], _C1, _C2, 0.0)
            nc.vector.cody_waite_cascade(r[:], r[:], k2[:], _C1, _C2, 0.0)
            nc.vector.cody_waite_cascade(r[:], r[:], k[:], _C3, 0.0, 0.0)
            yc = ap.tile([64, BS], F32, name="yc_emb")
            ys = ap.tile([64, BS], F32, name="ys_emb")
            nc.vector.add_range_wrap(yc[:], r[:], _HALF_PI_F, _PI_F, _TWO_PI_F)
            nc.vector.add_range_wrap(ys[:], r[:], 0.0, _PI_F, _TWO_PI_F)
            dT = ap.tile([64, BS], F32, name="dT")
            nc.scalar.activation(dT[0:32, :], yc[0:32, :], AF.Sin)
            nc.scalar.activation(dT[32:64, :], ys[32:64, :], AF.Sin)
            dTr = ap.tile([64, BS], F32R, name="dTr")
            nc.vector.tensor_copy(dTr[:], dT[:])

            # ---- helpers ----
            def rsqrt_nr(nm, vv):
                """rsqrt of (16,1) tile vv (v+eps already applied), ~1ulp."""
                gi = ap.tile([16, 1], I32, name=f"gi_{nm}")
                mg = ap.tile([16, 1], I32, name=f"mg_{nm}")
                nc.vector.memset(mg[:], 0x5F3759DF)
                nc.vector.tensor_scalar(gi[:], vv[:].bitcast(I32), 1, None,
                                        ALU.logical_shift_right)
                nc.vector.tensor_sub(gi[:], mg[:], gi[:])
                g = gi[:].bitcast(F32)
                tt = ap.tile([16, 1], F32, name=f"tt_{nm}")
                for _ in range(3):
                    nc.vector.tensor_mul(tt[:], g, g)
                    nc.vector.tensor_mul(tt[:], tt[:], vv[:])
                    nc.vector.tensor_scalar(tt[:], tt[:], -0.5, 1.5, ALU.mult, ALU.add)
                    nc.vector.tensor_mul(g, g, tt[:])
                return gi

            def ln_tanh(nm, pin, F, g_ap, be_ap):
                """LayerNorm(+affine)+tanh of PSUM (16,F) -> SBUF tile."""
                s = ap.tile([16, 1], F32, name=f"s_{nm}")
                scr = ap.tile([16, F], F32, name=f"scr_{nm}")
                nc.scalar.activation(scr[:], pin, AF.Identity, accum_out=s[:])
                nm_t = ap.tile([16, 1], F32, name=f"nm_{nm}")
                nc.vector.tensor_scalar(nm_t[:], s[:], -1.0 / F, None, ALU.mult)
                xc = ap.tile([16, F], F32, name=f"xc_{nm}")
                nc.scalar.activation(xc[:], pin, AF.Identity, bias=nm_t[:, 0:1])
                sq = ap.tile([16, F], F32, name=f"sq_{nm}")
                v = ap.tile([16, 1], F32, name=f"v_{nm}")
                nc.scalar.activation(sq[:], xc[:], AF.Square, accum_out=v[:])
                nc.vector.tensor_scalar(v[:], v[:], 1.0 / F, 1e-5, ALU.mult, ALU.add)
                rs = rsqrt_nr(nm, v)
                t2 = ap.tile([16, F], F32, name=f"t2_{nm}")
                nc.scalar.activation(t2[:], xc[:], AF.Identity,
                                     scale=rs[:, 0:1].bitcast(F32))
                nc.vector.tensor_mul(t2[:], t2[:], g_ap)
                nc.vector.tensor_add(t2[:], t2[:], be_ap)
                out = ap.tile([16, F], F32, name=f"act_{nm}")
                nc.scalar.activation(out[:], t2[:], AF.Tanh)
                return out

            def transpose_pack(nm, src, col_blocks, p_out, dt_out=F32):
                """Transpose (16, p_out) col blocks of src into one SBUF tile.

                col_blocks: list of (col_offset,) source column starts, each
                block is (16, p_out) -> out (p_out, 16) written at free
                offset i*16. Returns SBUF tile (p_out, 16*len(col_blocks)).
                """
                n = len(col_blocks)
                pt = tr_ps.tile([p_out, 16 * n], F32, name=f"pt_{nm}", tag="trp")
                for i, co in enumerate(col_blocks):
                    nc.tensor.transpose(pt[:, i * 16:(i + 1) * 16],
                                        src[:, co:co + p_out], idt[:])
                st = ap.tile([p_out, 16 * n], dt_out, name=f"st_{nm}")
                nc.vector.tensor_copy(st[:], pt[:])
                return st

            # ---- layer 1 ----
            ph = mlp_ps.tile([16, H1], F32, name="ph", tag="mlp")
            nc.tensor.matmul(ph[:], dTr[:], w1[:], start=True, stop=False)
            nc.tensor.matmul(ph[:], ones_ap, b1_ap, start=False, stop=True)
            h = ln_tanh("l1", ph[:], H1, g1_ap, be1_ap)

            # ---- layer 2 ----
            hT = transpose_pack("hT", h, [128 * i for i in range(4)], 128, F32R)
            pkv = mlp_ps.tile([16, D1], F32, name="pkv", tag="mlp")
            n2 = ((0, 512), (512, 768))
            for i in range(4):
                w2c = wp.tile([128, D1], F32R, name=f"w2_{i}", tag="wch")
                nc.sync.dma_start(out=w2c[:], in_=w2_d[128 * i:128 * (i + 1), :])
                for ns, ne in n2:
                    nc.tensor.matmul(pkv[:, ns:ne], hT[:, 16 * i:16 * (i + 1)],
                                     w2c[:, ns:ne], start=(i == 0), stop=False)
            for ns, ne in n2:
                nc.tensor.matmul(pkv[:, ns:ne], ones_ap, b2_ap[:, ns:ne],
                                 start=False, stop=True)
            kv = ln_tanh("l2", pkv[:], D1, g2_ap, be2_ap)

            nc.sync.dma_start(out=eq3[:], in_=eq3_d[:])
            nc.sync.dma_start(out=eq2[:], in_=eq2_d[:])
            nc.sync.dma_start(out=eq1[:], in_=eq1_d[:])

            # ---- gather tiles from kv ----
            g11 = transpose_pack("g11", kv, [64 * c for c in range(4)], 64, F32R)
            g22 = transpose_pack("g22", kv, [256 + 32 * c for c in range(8)], 32, F32R)
            g33 = transpose_pack("g33", kv, [512 + 16 * c for c in range(16)], 16, F32R)

            # ---- generic gather + mirror-expansion + store ----
            def gather_out(nm, dram, lhsT, eq, L, grp, eng="v"):
                """One group (<=128 rows) of one output.

                lhsT: (L, Mg) SBUF AP (f32r). eq: (L, L*L) one-hot quadrant
                matrix (f32r). Output rows dram[grp*128:grp*128+Mg].
                PSUM tiles span up to 2 banks (1024 f32) to amortize the
                PSUM->SBUF copy overhead.
                """
                Mg = lhsT.shape[1]
                Q = L * L
                PCH = min(1024, Q)             # psum tile cols (<=2 banks)
                nps = Q // PCH                 # psum tiles per quadrant
                QRp = PCH // L                 # quadrant rows per psum tile
                r0 = grp * 128
                TL = 2 * L

                def copy2(dst, sd):
                    if eng == "v":
                        nc.vector.tensor_copy(dst, sd)
                    else:
                        nc.scalar.activation(dst, sd, AF.Copy)

                def expand(st3, p3, dst_lr0, rows_rev):
                    """4-way mirror copy of one psum tile into staging rows."""
                    if rows_rev:
                        src_dir = p3[:, ::-1, :]
                        src_rev = p3[:, ::-1, ::-1]
                    else:
                        src_dir = p3[:, :, :]
                        src_rev = p3[:, :, ::-1]
                    dsl = st3[:, dst_lr0:dst_lr0 + QRp, :]
                    copy2(dsl[:, :, L:TL], src_dir)
                    copy2(dsl[:, :, 0:L], src_rev)

                for pi in range(nps):
                    pt = g_ps.tile([Mg, PCH], F32, name=f"gp_{nm}_{pi}", tag="gp")
                    for j in range(0, PCH, 512):
                        je = min(PCH, j + 512)
                        nc.tensor.matmul(pt[:, j:je], lhsT,
                                         eq[:, pi * PCH + j:pi * PCH + je],
                                         start=True, stop=True)
                    p3 = pt[:].rearrange("p (r c) -> p r c", r=QRp)
                    qlo = pi * QRp
                    if nps == 1:
                        # whole quadrant in one psum tile: one staging tile
                        st = sp.tile([Mg, 2 * L * TL], F32,
                                     name=f"sg_{nm}", tag="stage")
                        st3 = st[:].rearrange("p (r c) -> p r c", r=2 * L)
                        expand(st3, p3, 0, True)       # top half rows 0..L-1
                        expand(st3, p3, L, False)      # bottom rows L..2L-1
                        nc.sync.dma_start(
                            out=dram[r0:r0 + Mg, 0:2 * L * TL], in_=st[0:Mg, :])
                    else:
                        stb = sp.tile([Mg, QRp * TL], F32,
                                      name=f"sgb_{nm}_{pi}", tag="stage")
                        st3 = stb[:].rearrange("p (r c) -> p r c", r=QRp)
                        expand(st3, p3, 0, False)
                        f0 = L + qlo
                        nc.sync.dma_start(
                            out=dram[r0:r0 + Mg, f0 * TL:(f0 + QRp) * TL],
                            in_=stb[0:Mg, :])
                        stt = sp.tile([Mg, QRp * TL], F32,
                                      name=f"sgt_{nm}_{pi}", tag="stage")
                        st3 = stt[:].rearrange("p (r c) -> p r c", r=QRp)
                        expand(st3, p3, 0, True)
                        f0t = L - qlo - QRp
                        nc.sync.dma_start(
                            out=dram[r0:r0 + Mg, f0t * TL:(f0t + QRp) * TL],
                            in_=stt[0:Mg, :])

            # ---- layer 3 ----
            kvT = transpose_pack("kvT", kv, [128 * i for i in range(6)], 128, F32R)
            pkvb = mlp_ps.tile([16, D2], F32, name="pkvb", tag="mlp")
            n3 = ((0, 512), (512, 1024), (1024, 1536))
            for i in range(6):
                w3c = wp.tile([128, D2], F32R, name=f"w3_{i}", tag="wch")
                nc.sync.dma_start(out=w3c[:], in_=w3_d[128 * i:128 * (i + 1), :])
                for ns, ne in n3:
                    nc.tensor.matmul(pkvb[:, ns:ne], kvT[:, 16 * i:16 * (i + 1)],
                                     w3c[:, ns:ne], start=(i == 0), stop=False)
            for ns, ne in n3:
                nc.tensor.matmul(pkvb[:, ns:ne], ones_ap, b3_ap[:, ns:ne],
                                 start=False, stop=True)
            kvb = ln_tanh("l3", pkvb[:], D2, g3_ap, be3_ap)

            # ---- gather tiles from kvb ----
            gc1 = transpose_pack("gc1", kvb, [64 * c for c in range(8)], 64, F32R)
            gc2 = transpose_pack("gc2", kvb, [512 + 32 * c for c in range(16)], 32, F32R)
            gc3 = transpose_pack("gc3", kvb, [1024 + 16 * c for c in range(32)], 16, F32R)

            # ---- gathers: c1 first (largest), kv-based fill in ----
            gather_out("c1", c1_d, gc1[:, 0:128], eq1[:], 64, 0)
            gather_out("c11", c11_d, g11[:, 0:64], eq1[:], 64, 0)
            gather_out("c2a", c2_d, gc2[:, 0:128], eq2[:], 32, 0, eng="s")
            gather_out("c2b", c2_d, gc2[:, 128:256], eq2[:], 32, 1, eng="s")
            gather_out("c22", c22_d, g22[:, 0:128], eq2[:], 32, 0)
            for g in range(4):
                gather_out(f"c3{g}", c3_d, gc3[:, 128 * g:128 * (g + 1)], eq3[:], 16, g, eng="s")
            gather_out("c33a", c33_d, g33[:, 0:128], eq3[:], 16, 0, eng="s")
            gather_out("c33b", c33_d, g33[:, 128:256], eq3[:], 16, 1, eng="s")

    nc.compile()
    return nc


_CACHE = {}


def _get_nc():
    if "nc" not in _CACHE:
        _CACHE["nc"] = _build()
    return _CACHE["nc"]


def make_in_maps(x, W1, b1, g1, beta1, W2, b2, g2, beta2, W3, b3, g3, beta3):
    f = np.float32
    x = np.asarray(x, f)
    W1 = np.ascontiguousarray(np.asarray(W1, f))
    W2 = np.ascontiguousarray(np.asarray(W2, f))
    W3 = np.ascontiguousarray(np.asarray(W3, f))

    fb = _freq_bands()
    freqs = np.concatenate([fb, fb]).reshape(64, 1).astype(f)

    def rep16(v):
        return np.ascontiguousarray(np.broadcast_to(np.asarray(v, f)[None, :], (16, len(np.asarray(v)))))
    gmaps = {"g1p": rep16(g1), "be1p": rep16(beta1), "g2p": rep16(g2),
             "be2p": rep16(beta2), "g3p": rep16(g3), "be3p": rep16(beta3)}

    bp = np.zeros((1, 16 + H1 + D1 + D2), f)
    bp[0, 0:16] = 1.0
    bp[0, 16:16 + H1] = np.asarray(b1, f)
    bp[0, 16 + H1:16 + H1 + D1] = np.asarray(b2, f)
    bp[0, 16 + H1 + D1:] = np.asarray(b3, f)

    ident = np.eye(16, dtype=f)
    eq1, eq2, eq3 = _eq_mat(64), _eq_mat(32), _eq_mat(16)

    in_maps = []
    for i in range(NCORES):
        xs = x[i * BS:(i + 1) * BS, 0]
        in_maps.append({
            "xb": np.ascontiguousarray(np.broadcast_to(xs[None, :], (64, BS))),
            "freqs": freqs, "W1": W1, "W2": W2, "W3": W3,
            "biaspack": bp, "ident": ident,
            "eq1": eq1, "eq2": eq2, "eq3": eq3, **gmaps,
        })
    return in_maps


def assemble(res):
    def asm(key, CH, L):
        full = np.concatenate(
            [r[key].reshape(CH, BS, 2 * L, 2 * L).transpose(1, 0, 2, 3)
             for r in res], axis=0)
        return np.ascontiguousarray(full)

    return (asm("c1o", 8, 64), asm("c2o", 16, 32), asm("c3o", 32, 16),
            asm("c11o", 4, 64), asm("c22o", 8, 32), asm("c33o", 16, 16))


def kernel(**inputs):
    nc = _get_nc()
    in_maps = make_in_maps(**inputs)
    res = run_bass_kernel_spmd(nc, in_maps, list(range(NCORES))).results
    return assemble(res)


# revision 24
# speedup vs baseline: 1.0345x; 1.0045x over previous
"""Trainium2 Bass kernel for nn_Distance_kernel (histogram_binning).

Self-contained: takes FULL inputs, shards batch across 8 NeuronCores,
runs a Bass/Tile kernel per core, gathers the full outputs.

Per-core plan (bs = 16 batch rows):
  embed (cos/sin with split-k Cody-Waite range reduction + ACT Sin)
  -> 3-layer MLP (fp32 PE matmuls, LN via DVE + NR-rsqrt, ACT tanh)
  -> histogram gathers as one-hot QUADRANT matmuls (exact 4-fold symmetry
     of the radial bin map), mirror-expanded to full maps with
     negative-stride DVE copies, streamed out with large DMAs.
"""
import contextlib

import numpy as np

import concourse.bacc as bacc
import concourse.mybir as mybir
from concourse.tile import TileContext
from concourse.bass_utils import run_bass_kernel_spmd

F32 = mybir.dt.float32
F32R = mybir.dt.float32r
I32 = mybir.dt.int32
AF = mybir.ActivationFunctionType
ALU = mybir.AluOpType
AX = mybir.AxisListType

# ---------------- problem constants (hardcoded per spec) ----------------
B = 128
NCORES = 8
BS = B // NCORES            # 16
N_EMBED = 64
FE = 32                     # F_EMBED
H1 = 512
D1 = 768
D2 = 1536
WL = 0.638
PITCH = 8e-06

# (name, L, n_grp, Mg_last, channels) ; rows of output = CH*BS (c-major)
#   c1:  kvb[:, 0:512]    CH=8  L=64
#   c2:  kvb[:, 512:1024] CH=16 L=32
#   c3:  kvb[:, 1024:1536]CH=32 L=16
#   c11: kv[:, 0:256]     CH=4  L=64
#   c22: kv[:, 256:512]   CH=8  L=32
#   c33: kv[:, 512:768]   CH=16 L=16

# ---------------- embed / range-reduction constants ----------------
_TWO_PI = 2.0 * np.pi
_C1 = 6.28125                      # 2pi to ~10 bits (804/128)
_rem = _TWO_PI - _C1
_e = np.frexp(_rem)[1]
_C2 = float(np.round(_rem * 2.0 ** (12 - _e)) * 2.0 ** (_e - 12))
_C3 = float(np.float32(_TWO_PI - _C1 - _C2))
_INV2PI = float(np.float32(1.0 / _TWO_PI))
_MAGIC = float(np.float32(1.5 * 2 ** 23))
_PI_F = float(np.float32(np.pi))
_TWO_PI_F = float(np.float32(_TWO_PI))
_HALF_PI_F = float(np.float32(np.pi / 2))


def _freq_bands():
    """Exact f32 replication of the reference freq_bands computation."""
    wavelength = WL * 1e-06
    min_fre = 2 * np.pi / wavelength * (1 - 2 * (wavelength / PITCH / 2) ** 2) ** 0.5
    max_fre = 2 * np.pi / wavelength
    lin = np.linspace(1.0, FE, FE, dtype=np.float32)
    return (np.float32((max_fre - min_fre) / FE) * lin
            + np.float32(min_fre)).astype(np.float32)


def _idx_full(L):
    ax = np.linspace(-float(L), float(L), 2 * L)
    xg, yg = np.meshgrid(ax, ax, indexing="ij")
    dis = np.sqrt(xg ** 2 + yg ** 2)
    interval = dis.max() / L
    idx = np.floor(dis / (interval + 1e-4)).astype(np.int64)
    return idx


def _eq_mat(L):
    """One-hot expansion matrix for the positive quadrant: (L, L*L) f32."""
    idx = _idx_full(L)
    q = idx[L:, L:].ravel()
    assert q.max() < L
    E = np.zeros((L, L * L), np.float32)
    E[q, np.arange(L * L)] = 1.0
    return E


# ---------------- kernel builder ----------------

def _build():
    nc = bacc.Bacc("TRN2", target_bir_lowering=False, debug=False)

    # inputs
    xb_d = nc.declare_dram_parameter("xb", [64, BS], F32, isOutput=False)
    fr_d = nc.declare_dram_parameter("freqs", [64, 1], F32, isOutput=False)
    w1_d = nc.declare_dram_parameter("W1", [64, H1], F32R, isOutput=False)
    w2_d = nc.declare_dram_parameter("W2", [H1, D1], F32R, isOutput=False)
    w3_d = nc.declare_dram_parameter("W3", [D1, D2], F32R, isOutput=False)
    gb_ds = {}
    for nm_, F_ in (("g1p", H1), ("be1p", H1), ("g2p", D1), ("be2p", D1),
                    ("g3p", D2), ("be3p", D2)):
        gb_ds[nm_] = nc.declare_dram_parameter(nm_, [16, F_], F32, isOutput=False)
    bp_d = nc.declare_dram_parameter("biaspack", [1, 16 + H1 + D1 + D2], F32R, isOutput=False)
    id_d = nc.declare_dram_parameter("ident", [16, 16], F32, isOutput=False)
    eq1_d = nc.declare_dram_parameter("eq1", [64, 64 * 64], F32R, isOutput=False)
    eq2_d = nc.declare_dram_parameter("eq2", [32, 32 * 32], F32R, isOutput=False)
    eq3_d = nc.declare_dram_parameter("eq3", [16, 16 * 16], F32R, isOutput=False)

    # outputs: rows are (channel*BS + b), cols are flattened (2L, 2L)
    c1_d = nc.declare_dram_parameter("c1o", [128, 16384], F32, isOutput=True)
    c2_d = nc.declare_dram_parameter("c2o", [256, 4096], F32, isOutput=True)
    c3_d = nc.declare_dram_parameter("c3o", [512, 1024], F32, isOutput=True)
    c11_d = nc.declare_dram_parameter("c11o", [64, 16384], F32, isOutput=True)
    c22_d = nc.declare_dram_parameter("c22o", [128, 4096], F32, isOutput=True)
    c33_d = nc.declare_dram_parameter("c33o", [256, 1024], F32, isOutput=True)

    with TileContext(nc) as tc:
        with (
            tc.tile_pool(name="consts", bufs=1) as cp,
            tc.tile_pool(name="acts", bufs=1) as ap,
            tc.tile_pool(name="wch", bufs=3) as wp,
            tc.tile_pool(name="stage", bufs=4) as sp,
        ):
            tr_stack = contextlib.ExitStack()
            tr_ps = tr_stack.enter_context(
                tc.tile_pool(name="tr_ps", bufs=2, space="PSUM"))
            mlp_stack = contextlib.ExitStack()
            mlp_ps = mlp_stack.enter_context(
                tc.tile_pool(name="mlp_ps", bufs=1, space="PSUM"))
            # ---- constant loads ----
            xb = cp.tile([64, BS], F32, name="xb_t")
            fr = cp.tile([64, 1], F32, name="fr_t")
            w1 = cp.tile([64, H1], F32R, name="w1_t")
            gb_ts = {}
            for nm_, F_ in (("g1p", H1), ("be1p", H1), ("g2p", D1), ("be2p", D1),
                            ("g3p", D2), ("be3p", D2)):
                gb_ts[nm_] = cp.tile([16, F_], F32, name=nm_ + "_t")
            bp = cp.tile([1, 16 + H1 + D1 + D2], F32R, name="bp_t")
            idt = cp.tile([16, 16], F32, name="idt_t")
            eq1 = cp.tile([64, 64 * 64], F32R, name="eq1_t")
            eq2 = cp.tile([32, 32 * 32], F32R, name="eq2_t")
            eq3 = cp.tile([16, 16 * 16], F32R, name="eq3_t")
            nc.sync.dma_start(out=xb[:], in_=xb_d[:])
            nc.sync.dma_start(out=fr[:], in_=fr_d[:])
            nc.sync.dma_start(out=w1[:], in_=w1_d[:])
            for nm_ in gb_ts:
                nc.sync.dma_start(out=gb_ts[nm_][:], in_=gb_ds[nm_][:])
            nc.sync.dma_start(out=bp[:], in_=bp_d[:])
            nc.sync.dma_start(out=idt[:], in_=id_d[:])

            ones_ap = bp[:, 0:16]                       # (1,16)
            b1_ap = bp[:, 16:16 + H1]
            b2_ap = bp[:, 16 + H1:16 + H1 + D1]
            b3_ap = bp[:, 16 + H1 + D1:16 + H1 + D1 + D2]
            g1_ap, be1_ap = gb_ts["g1p"][:], gb_ts["be1p"][:]
            g2_ap, be2_ap = gb_ts["g2p"][:], gb_ts["be2p"][:]
            g3_ap, be3_ap = gb_ts["g3p"][:], gb_ts["be3p"][:]

            # ---- embed: t = x*f (exact f32), reduce mod 2pi, sin table ----
            t = ap.tile([64, BS], F32, name="t_emb")
            nc.vector.tensor_scalar(t[:], xb[:], fr[:, 0:1], None, ALU.mult)
            k = ap.tile([64, BS], F32, name="k_emb")
            nc.vector.tensor_scalar(k[:], t[:], _INV2PI, None, ALU.mult)
            nc.vector.tensor_scalar(k[:], k[:], _MAGIC, _MAGIC, ALU.add, ALU.subtract)
            k1 = ap.tile([64, BS], F32, name="k1_emb")
            nc.vector.tensor_scalar(k1[:], k[:], 2.0 ** -11, _MAGIC, ALU.mult, ALU.add)
            nc.vector.tensor_scalar(k1[:], k1[:], _MAGIC, 2048.0, ALU.subtract, ALU.mult)
            k2 = ap.tile([64, BS], F32, name="k2_emb")
            nc.vector.tensor_sub(k2[:], k[:], k1[:])
            r = ap.tile([64, BS], F32, name="r_emb")
            nc.vector.cody_waite_cascade(r[:], t[:], k1[:], _C1, _C2, 0.0)
            nc.vector.cody_waite_cascade(r[:], r[:], k2[:], _C1, _C2, 0.0)
            nc.vector.cody_waite_cascade(r[:], r[:], k[:], _C3, 0.0, 0.0)
            yc = ap.tile([64, BS], F32, name="yc_emb")
            ys = ap.tile([64, BS], F32, name="ys_emb")
            nc.vector.add_range_wrap(yc[:], r[:], _HALF_PI_F, _PI_F, _TWO_PI_F)
            nc.vector.add_range_wrap(ys[:], r[:], 0.0, _PI_F, _TWO_PI_F)
            dT = ap.tile([64, BS], F32, name="dT")
            nc.scalar.activation(dT[0:32, :], yc[0:32, :], AF.Sin)
            nc.scalar.activation(dT[32:64, :], ys[32:64, :], AF.Sin)
            dTr = ap.tile([64, BS], F32R, name="dTr")
            nc.vector.tensor_copy(dTr[:], dT[:])

            # ---- helpers ----
            def rsqrt_nr(nm, vv):
                """rsqrt of (16,1) tile vv (v+eps already applied), ~1ulp."""
                gi = ap.tile([16, 1], I32, name=f"gi_{nm}")
                mg = ap.tile([16, 1], I32, name=f"mg_{nm}")
                nc.vector.memset(mg[:], 0x5F3759DF)
                nc.vector.tensor_scalar(gi[:], vv[:].bitcast(I32), 1, None,
                                        ALU.logical_shift_right)
                nc.vector.tensor_sub(gi[:], mg[:], gi[:])
                g = gi[:].bitcast(F32)
                tt = ap.tile([16, 1], F32, name=f"tt_{nm}")
                for _ in range(3):
                    nc.vector.tensor_mul(tt[:], g, g)
                    nc.vector.tensor_mul(tt[:], tt[:], vv[:])
                    nc.vector.tensor_scalar(tt[:], tt[:], -0.5, 1.5, ALU.mult, ALU.add)
                    nc.vector.tensor_mul(g, g, tt[:])
                return gi

            def ln_tanh(nm, pin, F, g_ap, be_ap):
                """LayerNorm(+affine)+tanh of PSUM (16,F) -> SBUF tile."""
                s = ap.tile([16, 1], F32, name=f"s_{nm}")
                scr = ap.tile([16, F], F32, name=f"scr_{nm}")
                nc.scalar.activation(scr[:], pin, AF.Identity, accum_out=s[:])
                nm_t = ap.tile([16, 1], F32, name=f"nm_{nm}")
                nc.vector.tensor_scalar(nm_t[:], s[:], -1.0 / F, None, ALU.mult)
                xc = ap.tile([16, F], F32, name=f"xc_{nm}")
                nc.scalar.activation(xc[:], pin, AF.Identity, bias=nm_t[:, 0:1])
                sq = ap.tile([16, F], F32, name=f"sq_{nm}")
                v = ap.tile([16, 1], F32, name=f"v_{nm}")
                nc.scalar.activation(sq[:], xc[:], AF.Square, accum_out=v[:])
                nc.vector.tensor_scalar(v[:], v[:], 1.0 / F, 1e-5, ALU.mult, ALU.add)
                rs = rsqrt_nr(nm, v)
                t2 = ap.tile([16, F], F32, name=f"t2_{nm}")
                nc.scalar.activation(t2[:], xc[:], AF.Identity,
                                     scale=rs[:, 0:1].bitcast(F32))
                nc.vector.tensor_mul(t2[:], t2[:], g_ap)
                nc.vector.tensor_add(t2[:], t2[:], be_ap)
                out = ap.tile([16, F], F32, name=f"act_{nm}")
                nc.scalar.activation(out[:], t2[:], AF.Tanh)
                return out

            def transpose_pack(nm, src, col_blocks, p_out, dt_out=F32):
                """Transpose (16, p_out) col blocks of src into one SBUF tile.

                col_blocks: list of (col_offset,) source column starts, each
                block is (16, p_out) -> out (p_out, 16) written at free
                offset i*16. Returns SBUF tile (p_out, 16*len(col_blocks)).
                """
                n = len(col_blocks)
                pt = tr_ps.tile([p_out, 16 * n], F32, name=f"pt_{nm}", tag="trp")
                for i, co in enumerate(col_blocks):
                    nc.tensor.transpose(pt[:, i * 16:(i + 1) * 16],
                                        src[:, co:co + p_out], idt[:])
                st = ap.tile([p_out, 16 * n], dt_out, name=f"st_{nm}")
                nc.vector.tensor_copy(st[:], pt[:])
                return st

            # ---- layer 1 ----
            ph = mlp_ps.tile([16, H1], F32, name="ph", tag="mlp")
            nc.tensor.matmul(ph[:], dTr[:], w1[:], start=True, stop=False)
            nc.tensor.matmul(ph[:], ones_ap, b1_ap, start=False, stop=True)
            h = ln_tanh("l1", ph[:], H1, g1_ap, be1_ap)

            # ---- layer 2 ----
            hT = transpose_pack("hT", h, [128 * i for i in range(4)], 128, F32R)
            pkv = mlp_ps.tile([16, D1], F32, name="pkv", tag="mlp")
            n2 = ((0, 512), (512, 768))
            for i in range(4):
                w2c = wp.tile([128, D1], F32R, name=f"w2_{i}", tag="wch")
                nc.sync.dma_start(out=w2c[:], in_=w2_d[128 * i:128 * (i + 1), :])
                for ns, ne in n2:
                    nc.tensor.matmul(pkv[:, ns:ne], hT[:, 16 * i:16 * (i + 1)],
                                     w2c[:, ns:ne], start=(i == 0), stop=False)
            for ns, ne in n2:
                nc.tensor.matmul(pkv[:, ns:ne], ones_ap, b2_ap[:, ns:ne],
                                 start=False, stop=True)
            kv = ln_tanh("l2", pkv[:], D1, g2_ap, be2_ap)

            nc.sync.dma_start(out=eq3[:], in_=eq3_d[:])
            nc.sync.dma_start(out=eq2[:], in_=eq2_d[:])
            nc.sync.dma_start(out=eq1[:], in_=eq1_d[:])

            # ---- gather tiles from kv ----
            g11 = transpose_pack("g11", kv, [64 * c for c in range(4)], 64, F32R)
            g22 = transpose_pack("g22", kv, [256 + 32 * c for c in range(8)], 32, F32R)
            g33 = transpose_pack("g33", kv, [512 + 16 * c for c in range(16)], 16, F32R)

            # ---- generic gather + mirror-expansion + store ----
            def gather_out(nm, dram, lhsT, eq, L, grp, eng, pool):
                """One group (<=128 rows) of one output.

                lhsT: (L, Mg) SBUF AP (f32r). eq: (L, L*L) one-hot quadrant
                matrix (f32r). Output rows dram[grp*128:grp*128+Mg].
                PSUM tiles span up to 2 banks (1024 f32) to amortize the
                PSUM->SBUF copy overhead.
                """
                Mg = lhsT.shape[1]
                Q = L * L
                PCH = min(1024, Q)             # psum tile cols (<=2 banks)
                nps = Q // PCH                 # psum tiles per quadrant
                QRp = PCH // L                 # quadrant rows per psum tile
                r0 = grp * 128
                TL = 2 * L

                def copy2(dst, sd):
                    if eng == "v":
                        nc.vector.tensor_copy(dst, sd)
                    else:
                        nc.scalar.activation(dst, sd, AF.Copy)

                def expand(st3, p3, dst_lr0, rows_rev):
                    """4-way mirror copy of one psum tile into staging rows."""
                    if rows_rev:
                        src_dir = p3[:, ::-1, :]
                        src_rev = p3[:, ::-1, ::-1]
                    else:
                        src_dir = p3[:, :, :]
                        src_rev = p3[:, :, ::-1]
                    dsl = st3[:, dst_lr0:dst_lr0 + QRp, :]
                    copy2(dsl[:, :, L:TL], src_dir)
                    copy2(dsl[:, :, 0:L], src_rev)

                for pi in range(nps):
                    pt = pool.tile([Mg, PCH], F32, name=f"gp_{nm}_{pi}", tag="gp")
                    for j in range(0, PCH, 512):
                        je = min(PCH, j + 512)
                        nc.tensor.matmul(pt[:, j:je], lhsT,
                                         eq[:, pi * PCH + j:pi * PCH + je],
                                         start=True, stop=True)
                    p3 = pt[:].rearrange("p (r c) -> p r c", r=QRp)
                    qlo = pi * QRp
                    if nps == 1:
                        # whole quadrant in one psum tile: one staging tile
                        st = sp.tile([Mg, 2 * L * TL], F32,
                                     name=f"sg_{nm}", tag="stage")
                        st3 = st[:].rearrange("p (r c) -> p r c", r=2 * L)
                        expand(st3, p3, 0, True)       # top half rows 0..L-1
                        expand(st3, p3, L, False)      # bottom rows L..2L-1
                        nc.sync.dma_start(
                            out=dram[r0:r0 + Mg, 0:2 * L * TL], in_=st[0:Mg, :])
                    else:
                        stb = sp.tile([Mg, QRp * TL], F32,
                                      name=f"sgb_{nm}_{pi}", tag="stage")
                        st3 = stb[:].rearrange("p (r c) -> p r c", r=QRp)
                        expand(st3, p3, 0, False)
                        f0 = L + qlo
                        nc.sync.dma_start(
                            out=dram[r0:r0 + Mg, f0 * TL:(f0 + QRp) * TL],
                            in_=stb[0:Mg, :])
                        stt = sp.tile([Mg, QRp * TL], F32,
                                      name=f"sgt_{nm}_{pi}", tag="stage")
                        st3 = stt[:].rearrange("p (r c) -> p r c", r=QRp)
                        expand(st3, p3, 0, True)
                        f0t = L - qlo - QRp
                        nc.sync.dma_start(
                            out=dram[r0:r0 + Mg, f0t * TL:(f0t + QRp) * TL],
                            in_=stt[0:Mg, :])

            # ---- layer 3 ----
            kvT = transpose_pack("kvT", kv, [128 * i for i in range(6)], 128, F32R)
            pkvb = mlp_ps.tile([16, D2], F32, name="pkvb", tag="mlp")
            n3 = ((0, 512), (512, 1024), (1024, 1536))
            for i in range(6):
                w3c = wp.tile([128, D2], F32R, name=f"w3_{i}", tag="wch")
                nc.sync.dma_start(out=w3c[:], in_=w3_d[128 * i:128 * (i + 1), :])
                for ns, ne in n3:
                    nc.tensor.matmul(pkvb[:, ns:ne], kvT[:, 16 * i:16 * (i + 1)],
                                     w3c[:, ns:ne], start=(i == 0), stop=False)
            for ns, ne in n3:
                nc.tensor.matmul(pkvb[:, ns:ne], ones_ap, b3_ap[:, ns:ne],
                                 start=False, stop=True)
            kvb = ln_tanh("l3", pkvb[:], D2, g3_ap, be3_ap)
            mlp_stack.close()

            # ---- gather tiles from kvb ----
            gc1 = transpose_pack("gc1", kvb, [64 * c for c in range(8)], 64, F32R)
            gc2 = transpose_pack("gc2", kvb, [512 + 32 * c for c in range(16)], 32, F32R)
            gc3 = transpose_pack("gc3", kvb, [1024 + 16 * c for c in range(32)], 16, F32R)

            tr_stack.close()
            g_stack = contextlib.ExitStack()
            gv_ps = g_stack.enter_context(
                tc.tile_pool(name="gv_ps", bufs=2, space="PSUM"))
            gs_ps = g_stack.enter_context(
                tc.tile_pool(name="gs_ps", bufs=2, space="PSUM"))

            # ---- gathers: two independent copy streams (DVE / ACT) ----
            gather_out("c1", c1_d, gc1[:, 0:128], eq1[:], 64, 0, "v", gv_ps)
            gather_out("c2a", c2_d, gc2[:, 0:128], eq2[:], 32, 0, "s", gs_ps)
            gather_out("c11", c11_d, g11[:, 0:64], eq1[:], 64, 0, "v", gv_ps)
            gather_out("c2b", c2_d, gc2[:, 128:256], eq2[:], 32, 1, "s", gs_ps)
            gather_out("c22", c22_d, g22[:, 0:128], eq2[:], 32, 0, "v", gv_ps)
            for g in range(4):
                gather_out(f"c3{g}", c3_d, gc3[:, 128 * g:128 * (g + 1)], eq3[:], 16, g, "s", gs_ps)
            gather_out("c33a", c33_d, g33[:, 0:128], eq3[:], 16, 0, "s", gs_ps)
            gather_out("c33b", c33_d, g33[:, 128:256], eq3[:], 16, 1, "s", gs_ps)
            g_stack.close()

    nc.compile()
    return nc


_CACHE = {}


def _get_nc():
    if "nc" not in _CACHE:
        _CACHE["nc"] = _build()
    return _CACHE["nc"]


def make_in_maps(x, W1, b1, g1, beta1, W2, b2, g2, beta2, W3, b3, g3, beta3):
    f = np.float32
    x = np.asarray(x, f)
    W1 = np.ascontiguousarray(np.asarray(W1, f))
    W2 = np.ascontiguousarray(np.asarray(W2, f))
    W3 = np.ascontiguousarray(np.asarray(W3, f))

    fb = _freq_bands()
    freqs = np.concatenate([fb, fb]).reshape(64, 1).astype(f)

    def rep16(v):
        return np.ascontiguousarray(np.broadcast_to(np.asarray(v, f)[None, :], (16, len(np.asarray(v)))))
    gmaps = {"g1p": rep16(g1), "be1p": rep16(beta1), "g2p": rep16(g2),
             "be2p": rep16(beta2), "g3p": rep16(g3), "be3p": rep16(beta3)}

    bp = np.zeros((1, 16 + H1 + D1 + D2), f)
    bp[0, 0:16] = 1.0
    bp[0, 16:16 + H1] = np.asarray(b1, f)
    bp[0, 16 + H1:16 + H1 + D1] = np.asarray(b2, f)
    bp[0, 16 + H1 + D1:] = np.asarray(b3, f)

    ident = np.eye(16, dtype=f)
    eq1, eq2, eq3 = _eq_mat(64), _eq_mat(32), _eq_mat(16)

    in_maps = []
    for i in range(NCORES):
        xs = x[i * BS:(i + 1) * BS, 0]
        in_maps.append({
            "xb": np.ascontiguousarray(np.broadcast_to(xs[None, :], (64, BS))),
            "freqs": freqs, "W1": W1, "W2": W2, "W3": W3,
            "biaspack": bp, "ident": ident,
            "eq1": eq1, "eq2": eq2, "eq3": eq3, **gmaps,
        })
    return in_maps


def assemble(res):
    def asm(key, CH, L):
        full = np.concatenate(
            [r[key].reshape(CH, BS, 2 * L, 2 * L).transpose(1, 0, 2, 3)
             for r in res], axis=0)
        return np.ascontiguousarray(full)

    return (asm("c1o", 8, 64), asm("c2o", 16, 32), asm("c3o", 32, 16),
            asm("c11o", 4, 64), asm("c22o", 8, 32), asm("c33o", 16, 16))


def kernel(**inputs):
    nc = _get_nc()
    in_maps = make_in_maps(**inputs)
    res = run_bass_kernel_spmd(nc, in_maps, list(range(NCORES))).results
    return assemble(res)
